# revision 12
# baseline (speedup 1.0000x reference)
"""MoE router layer (nn_ControllerLayer) on 8 Trainium2 NeuronCores.

Reference computation (per batch b of 8, S=1024 rows, D=E=1024):
    logits = x @ W.T            [B, S, E]
    probs  = softmax(logits)
    p, idx = top2(probs)
    y      = p0 * x[b, idx0] + p1 * x[b, idx1]
    aux    = 0.01 * E * sum(mean_probs * bincount(idx)/sum)

Sharding: data-parallel over the batch dim — core c gets x[c] and a
replica of W. Since E == S, the top-2 "expert gather" is a row gather
from the same core's x, done with indirect DMA. Aux-loss reductions
return per-core partials ([E] prob column sums, top-2 indices) that the
host combines (psum across devices, done on host since outputs are
gathered anyway).

The logits matmul carries the only real precision constraint: top-2
selection must match the fp32 reference (a flipped near-tie makes that
whole output row wrong). Plain bf16 flips ~80 rows; fp32 runs at 4
cycles/row on the PE. Instead x and W are split into bf16 hi/lo pairs
(x = xh + xl, W = wh + wl) and logits = xh@wh + xl@wh + xh@wl — three
full-rate bf16 matmuls with ~2e-5 absolute logit error (verified: zero
top-2 flips vs the fp32 reference on the seed-0 inputs; min top-2/3
margin is 1.4e-6, max split error 2.3e-5... the margin distribution has
P(margin < 1e-4) ~ 2e-4 so the margin-vs-error gap holds generically).
The split and the D-major transpose (contraction dim on partitions)
are host-side input marshalling, so the device runs no transposes.

Per-core kernel:
  1. 3-term bf16 matmul -> logits in PSUM (fp32 accumulate).
  2. exp straight out of PSUM (ScalarE) -> bf16 u + fp32 row sums.
  3. top-8 values + indices off the PSUM logits (VectorE max/max_index).
  4. indirect-DMA gather of the two selected x rows; y = p0*g0 + p1*g1.
     The gather+combine stage is software-pipelined one tile behind the
     matmul stage so gather latency never blocks the max_index that
     frees the next PSUM slot.
  5. probs column sums via a [1x128]@[128xE] ones-matmul with per-row
     1/sum as weights (bf16), interleaved into the matmul stream.
"""
import sys

if "/opt/trn_rl_repo" not in sys.path:
    sys.path.insert(0, "/opt/trn_rl_repo")

import numpy as np
import ml_dtypes

import concourse.bacc as bacc
import concourse.bass as bass
import concourse.mybir as mybir
from concourse import bass_utils
from concourse.tile import TileContext

P = 128
B, S, D = 8, 1024, 1024
E = 1024
K = 2
ALPHA = 0.01
T = S // P  # 8 row tiles
NCORES = 8

f32 = mybir.dt.float32
bf16 = mybir.dt.bfloat16
u32 = mybir.dt.uint32


def _build():
    nc = bacc.Bacc("TRN2", target_bir_lowering=False, debug=False,
                   num_devices=NCORES)

    x_d = nc.dram_tensor("x", (S, D), f32, kind="ExternalInput").ap()
    xh_d = nc.dram_tensor("xh", (D, S), bf16, kind="ExternalInput").ap()
    xl_d = nc.dram_tensor("xl", (D, S), bf16, kind="ExternalInput").ap()
    wh_d = nc.dram_tensor("wh", (D, E), bf16, kind="ExternalInput").ap()
    wl_d = nc.dram_tensor("wl", (D, E), bf16, kind="ExternalInput").ap()
    y_d = nc.dram_tensor("y", (S, D), f32, kind="ExternalOutput").ap()
    ps_d = nc.dram_tensor("ps", (1, E), f32, kind="ExternalOutput").ap()
    idx_d = nc.dram_tensor("idx2", (S, K), u32, kind="ExternalOutput").ap()

    views = {n: a.rearrange("(po pi) f -> pi po f", pi=P)
             for n, a in (("xh", xh_d), ("xl", xl_d), ("wh", wh_d), ("wl", wl_d))}

    with TileContext(nc) as tc:
        with (
            tc.tile_pool(name="big", bufs=1) as big,
            tc.tile_pool(name="work", bufs=4) as work,
            tc.tile_pool(name="stats", bufs=1) as stats,
            tc.tile_pool(name="psum", bufs=3, space="PSUM") as ps,
            tc.tile_pool(name="psaux", bufs=1, space="PSUM") as psx,
        ):
            # D-major matmul operands, loaded per k-tile so the first
            # matmuls can start after ~1MB of DMA; two HWDGE queues
            # (sync + scalar) to get past the single-queue ~284 GB/s.
            ops = {n: big.tile([P, T, S], bf16, tag=n, name=n) for n in views}
            for k in range(0, T, 2):
                for n in ("xh", "wh"):
                    nc.sync.dma_start(ops[n][:, k:k + 2, :], views[n][:, k:k + 2, :])
                for n in ("wl", "xl"):
                    nc.scalar.dma_start(ops[n][:, k:k + 2, :], views[n][:, k:k + 2, :])
            xh_t, xl_t = ops["xh"], ops["xl"]
            wh_t, wl_t = ops["wh"], ops["wl"]

            u_bf = big.tile([P, T, E], bf16, tag="u")
            acc = stats.tile([P, T], f32)     # per-row sum(exp(logits))
            rc = stats.tile([P, T], f32)      # 1/acc
            rc_bf = stats.tile([P, T], bf16)

            # HAM warm-up: a few dummy matmuls while the first operand
            # tiles are still in flight, so MM starts at 2.4 GHz.
            dmy_w = stats.tile([P, 1], bf16)
            dmy_r = stats.tile([P, 512], bf16)
            nc.gpsimd.memset(dmy_w[:], 0)
            nc.gpsimd.memset(dmy_r[:], 0)
            aux_ps = psx.tile([1, E], f32, tag="aux")
            for j in range(8):
                nc.tensor.matmul(aux_ps[:1, (j % 2) * 512:(j % 2) * 512 + 512],
                                 lhsT=dmy_w[:], rhs=dmy_r[:],
                                 start=True, stop=True, skip_group_check=True)

            pending = []

            def combine(st):
                # y = p0*g0 + p1*g1 on DVE + GpSimd only: ScalarE must stay
                # free of gather-dependent work, else a waiting t1 in its
                # FIFO delays the next exp and the whole PE round with it
                g0, g1, p01, m = st
                yt = work.tile([P, D], f32, tag="yt")
                t1 = work.tile([P, D], f32, tag="t1")
                nc.vector.tensor_scalar_mul(yt[:], g0[:], p01[:, 0:1])
                nc.gpsimd.tensor_scalar_mul(t1[:], g1[:], p01[:, 1:2])
                nc.vector.tensor_add(yt[:], yt[:], t1[:])
                nc.sync.dma_start(y_d[m * P:(m + 1) * P, :], yt[:])

            def aux_mm(m):
                # probs column sums: 1/sum-weighted ones-matmul over u.
                # Interleaved into the MM stream; separate PSUM bank, so
                # the open accumulation group is hardware-safe.
                for nh in range(2):
                    nc.tensor.matmul(
                        aux_ps[:1, nh * 512:(nh + 1) * 512],
                        lhsT=rc_bf[:, m:m + 1],
                        rhs=u_bf[:, m, nh * 512:(nh + 1) * 512],
                        start=(m == 0), stop=(m == T - 1),
                        skip_group_check=True)

            # (x lo?, w lo?) split terms — xh stationary twice in a row so
            # the PE only loads 2 distinct weight sets per k-slice
            PAIRS = ((0, 0), (0, 1), (1, 0))

            for m in range(T):
                lg = ps.tile([P, E], f32, tag="lg")
                mslc = slice(m * P, (m + 1) * P)
                # k outer, split-term middle, both N-halves inner so each
                # stationary x-slice is loaded once for two matmuls.
                for k in range(T):
                    for pi, (xlo, wlo) in enumerate(PAIRS):
                        lhs = xl_t if xlo else xh_t
                        rhs = wl_t if wlo else wh_t
                        for nh in range(2):
                            nslc = slice(nh * 512, (nh + 1) * 512)
                            nc.tensor.matmul(
                                lg[:, nslc],
                                lhsT=lhs[:, k, mslc],
                                rhs=rhs[:, k, nslc],
                                start=(k == 0 and pi == 0),
                                stop=(k == T - 1 and pi == len(PAIRS) - 1),
                            )
                    if k == 6 and m > 0:
                        aux_mm(m - 1)

                # top-8 logits + indices straight off PSUM, and the row
                # gathers they gate — before the softmax-sum chain, so
                # every round's gather DMA launches as early as possible
                t8 = work.tile([P, 8], f32, tag="t8")
                i8 = work.tile([P, 8], u32, tag="i8")
                nc.vector.max(out=t8[:], in_=lg[:])
                nc.vector.max_index(out=i8[:], in_max=t8[:], in_values=lg[:])
                nc.sync.dma_start(idx_d[m * P:(m + 1) * P, :], i8[:, 0:K])

                g0 = work.tile([P, D], f32, tag="g0")
                g1 = work.tile([P, D], f32, tag="g1")
                nc.gpsimd.indirect_dma_start(
                    out=g0[:], out_offset=None, in_=x_d[:],
                    in_offset=bass.IndirectOffsetOnAxis(ap=i8[:, 0:1], axis=0))
                nc.gpsimd.indirect_dma_start(
                    out=g1[:], out_offset=None, in_=x_d[:],
                    in_offset=bass.IndirectOffsetOnAxis(ap=i8[:, 1:2], axis=0))

                # u = exp(logits) (bf16) + fp32 row sums
                nc.scalar.activation(u_bf[:, m, :], lg[:],
                                     mybir.ActivationFunctionType.Exp,
                                     accum_out=acc[:, m:m + 1])
                nc.vector.reciprocal(rc[:, m:m + 1], acc[:, m:m + 1])
                # bf16 copy for the aux ones-matmul weights, produced on
                # DVE right behind the fp32 reciprocal so the interleaved
                # aux matmul never waits on a late-positioned ScalarE op
                nc.vector.tensor_copy(rc_bf[:, m:m + 1], rc[:, m:m + 1])

                # p_k = exp(top_k) / sum
                p01 = work.tile([P, K], f32, tag="p01")
                nc.scalar.activation(p01[:], t8[:, 0:K],
                                     mybir.ActivationFunctionType.Exp)
                nc.vector.tensor_scalar_mul(p01[:], p01[:], rc[:, m:m + 1])

                # two-tile-deep pipeline: gathers get ~2 matmul rounds of
                # slack before anything on a compute engine waits on them
                pending.append((g0, g1, p01, m))
                if len(pending) > 2:
                    combine(pending.pop(0))

            aux_mm(T - 1)
            for st in pending:
                combine(st)
            ps_sb = stats.tile([1, E], f32)
            nc.vector.tensor_copy(ps_sb[:], aux_ps[:])
            nc.sync.dma_start(ps_d[:], ps_sb[:])

    nc.finalize()
    return nc


_NC = None


def _split_T(a):
    """Return (hi, lo) bf16 split of a.T (fp32 [R, C] -> two [C, R])."""
    at = np.ascontiguousarray(a.T)
    hi = at.astype(ml_dtypes.bfloat16)
    lo = (at - hi.astype(np.float32)).astype(ml_dtypes.bfloat16)
    return hi, lo


def _run(x, W, **kw):
    global _NC
    if _NC is None:
        _NC = _build()
    x = np.ascontiguousarray(np.asarray(x, dtype=np.float32))
    W = np.ascontiguousarray(np.asarray(W, dtype=np.float32))
    wh, wl = _split_T(W)
    in_maps = []
    for c in range(NCORES):
        xh, xl = _split_T(x[c])
        in_maps.append({"x": x[c], "xh": xh, "xl": xl, "wh": wh, "wl": wl})
    return bass_utils.run_bass_kernel_spmd(
        _NC, in_maps, core_ids=list(range(NCORES)), **kw)


def kernel(x, W):
    res = _run(x, W)
    outs = res.results
    y = np.stack([outs[c]["y"] for c in range(NCORES)], axis=0)

    probs_sum = np.zeros(E, dtype=np.float64)
    idx_all = []
    for c in range(NCORES):
        probs_sum += outs[c]["ps"].reshape(E).astype(np.float64)
        idx_all.append(outs[c]["idx2"].reshape(-1))
    router_probs = (probs_sum / (B * S)).astype(np.float32)
    counts = np.bincount(np.concatenate(idx_all), minlength=E).astype(np.float32)
    router_fraction = counts / counts.sum()
    aux = np.float32(ALPHA * E * np.sum(router_probs * router_fraction,
                                        dtype=np.float64))
    return y, aux


# revision 14
# speedup vs baseline: 1.7561x; 1.7561x over previous
"""MoE router layer (nn_ControllerLayer) on 8 Trainium2 NeuronCores.

Reference computation (per batch b of 8, S=1024 rows, D=E=1024):
    logits = x @ W.T            [B, S, E]
    probs  = softmax(logits)
    p, idx = top2(probs)
    y      = p0 * x[b, idx0] + p1 * x[b, idx1]
    aux    = 0.01 * E * sum(mean_probs * bincount(idx)/sum)

Sharding: data-parallel over the batch dim — core c gets x[c] and a
replica of W. Since E == S, the top-2 "expert gather" is a row gather
from the same core's x, done with indirect DMA. Aux-loss reductions
return per-core partials ([E] prob column sums, top-2 indices) that the
host combines (psum across devices, done on host since outputs are
gathered anyway).

The logits matmul carries the only real precision constraint: top-2
selection must match the fp32 reference (a flipped near-tie makes that
whole output row wrong). Plain bf16 flips ~80 rows; fp32 runs at 4
cycles/row on the PE. Instead x and W are split into bf16 hi/lo pairs
(x = xh + xl, W = wh + wl) and logits = xh@wh + xl@wh + xh@wl — three
full-rate bf16 matmuls with ~2e-5 absolute logit error (verified: zero
top-2 flips vs the fp32 reference on the seed-0 inputs; min top-2/3
margin is 1.4e-6, max split error 2.3e-5... the margin distribution has
P(margin < 1e-4) ~ 2e-4 so the margin-vs-error gap holds generically).
The split and the D-major transpose (contraction dim on partitions)
are host-side input marshalling, so the device runs no transposes.

Per-core kernel:
  1. 3-term bf16 matmul -> logits in PSUM (fp32 accumulate).
  2. exp straight out of PSUM (ScalarE) -> bf16 u + fp32 row sums.
  3. top-8 values + indices off the PSUM logits (VectorE max/max_index).
  4. indirect-DMA gather of the two selected x rows; y = p0*g0 + p1*g1.
     The gather+combine stage is software-pipelined one tile behind the
     matmul stage so gather latency never blocks the max_index that
     frees the next PSUM slot.
  5. probs column sums via a [1x128]@[128xE] ones-matmul with per-row
     1/sum as weights (bf16), interleaved into the matmul stream.
"""
import sys

if "/opt/trn_rl_repo" not in sys.path:
    sys.path.insert(0, "/opt/trn_rl_repo")

import numpy as np
import ml_dtypes

import concourse.bacc as bacc
import concourse.bass as bass
import concourse.mybir as mybir
from concourse import bass_utils
from concourse.tile import TileContext

P = 128
B, S, D = 8, 1024, 1024
E = 1024
K = 2
ALPHA = 0.01
T = S // P  # 8 row tiles
NCORES = 8

f32 = mybir.dt.float32
bf16 = mybir.dt.bfloat16
u32 = mybir.dt.uint32


def _build():
    nc = bacc.Bacc("TRN2", target_bir_lowering=False, debug=False,
                   num_devices=NCORES)

    x_d = nc.dram_tensor("x", (S, D), f32, kind="ExternalInput").ap()
    xh_d = nc.dram_tensor("xh", (D, S), bf16, kind="ExternalInput").ap()
    xl_d = nc.dram_tensor("xl", (D, S), bf16, kind="ExternalInput").ap()
    wh_d = nc.dram_tensor("wh", (D, E), bf16, kind="ExternalInput").ap()
    wl_d = nc.dram_tensor("wl", (D, E), bf16, kind="ExternalInput").ap()
    y_d = nc.dram_tensor("y", (S, D), f32, kind="ExternalOutput").ap()
    ps_d = nc.dram_tensor("ps", (1, E), f32, kind="ExternalOutput").ap()
    idx_d = nc.dram_tensor("idx2", (S, K), u32, kind="ExternalOutput").ap()

    views = {n: a.rearrange("(po pi) f -> pi po f", pi=P)
             for n, a in (("xh", xh_d), ("xl", xl_d), ("wh", wh_d), ("wl", wl_d))}

    with TileContext(nc) as tc:
        with (
            tc.tile_pool(name="big", bufs=1) as big,
            tc.tile_pool(name="work", bufs=4) as work,
            tc.tile_pool(name="stats", bufs=1) as stats,
            tc.tile_pool(name="psum", bufs=3, space="PSUM") as ps,
            tc.tile_pool(name="psaux", bufs=1, space="PSUM") as psx,
        ):
            # D-major matmul operands, loaded per k-tile so the first
            # matmuls can start after ~1MB of DMA; two HWDGE queues
            # (sync + scalar) to get past the single-queue ~284 GB/s.
            ops = {n: big.tile([P, T, S], bf16, tag=n, name=n) for n in views}
            for k in range(0, T, 2):
                for n in ("xh", "wh"):
                    nc.sync.dma_start(ops[n][:, k:k + 2, :], views[n][:, k:k + 2, :])
                for n in ("wl", "xl"):
                    nc.scalar.dma_start(ops[n][:, k:k + 2, :], views[n][:, k:k + 2, :])
            xh_t, xl_t = ops["xh"], ops["xl"]
            wh_t, wl_t = ops["wh"], ops["wl"]

            u_bf = big.tile([P, T, E], bf16, tag="u")
            acc = stats.tile([P, T], f32)     # per-row sum(exp(logits))
            rc = stats.tile([P, T], f32)      # 1/acc
            rc_bf = stats.tile([P, T], bf16)

            # HAM warm-up: a few dummy matmuls while the first operand
            # tiles are still in flight, so MM starts at 2.4 GHz.
            dmy_w = stats.tile([P, 1], bf16)
            dmy_r = stats.tile([P, 512], bf16)
            nc.gpsimd.memset(dmy_w[:], 0)
            nc.gpsimd.memset(dmy_r[:], 0)
            aux_ps = psx.tile([1, E], f32, tag="aux")
            for j in range(8):
                nc.tensor.matmul(aux_ps[:1, (j % 2) * 512:(j % 2) * 512 + 512],
                                 lhsT=dmy_w[:], rhs=dmy_r[:],
                                 start=True, stop=True, skip_group_check=True)

            pending = []

            def combine(st):
                # y = p0*g0 + p1*g1 on DVE + GpSimd only: ScalarE must stay
                # free of gather-dependent work, else a waiting t1 in its
                # FIFO delays the next exp and the whole PE round with it
                g0, g1, p01, m = st
                yt = work.tile([P, D], f32, tag="yt")
                t1 = work.tile([P, D], f32, tag="t1")
                nc.vector.tensor_scalar_mul(yt[:], g0[:], p01[:, 0:1])
                nc.vector.tensor_scalar_mul(t1[:], g1[:], p01[:, 1:2])
                nc.vector.tensor_add(yt[:], yt[:], t1[:])
                nc.sync.dma_start(y_d[m * P:(m + 1) * P, :], yt[:])

            def aux_mm(m):
                # probs column sums: 1/sum-weighted ones-matmul over u.
                # Interleaved into the MM stream; separate PSUM bank, so
                # the open accumulation group is hardware-safe.
                for nh in range(2):
                    nc.tensor.matmul(
                        aux_ps[:1, nh * 512:(nh + 1) * 512],
                        lhsT=rc_bf[:, m:m + 1],
                        rhs=u_bf[:, m, nh * 512:(nh + 1) * 512],
                        start=(m == 0), stop=(m == T - 1),
                        skip_group_check=True)

            # (x lo?, w lo?) split terms — xh stationary twice in a row so
            # the PE only loads 2 distinct weight sets per k-slice
            PAIRS = ((0, 0), (0, 1), (1, 0))

            for m in range(T):
                lg = ps.tile([P, E], f32, tag="lg")
                mslc = slice(m * P, (m + 1) * P)
                # k outer, split-term middle, both N-halves inner so each
                # stationary x-slice is loaded once for two matmuls.
                for k in range(T):
                    for pi, (xlo, wlo) in enumerate(PAIRS):
                        lhs = xl_t if xlo else xh_t
                        rhs = wl_t if wlo else wh_t
                        for nh in range(2):
                            nslc = slice(nh * 512, (nh + 1) * 512)
                            nc.tensor.matmul(
                                lg[:, nslc],
                                lhsT=lhs[:, k, mslc],
                                rhs=rhs[:, k, nslc],
                                start=(k == 0 and pi == 0),
                                stop=(k == T - 1 and pi == len(PAIRS) - 1),
                            )
                    if k == 6 and m > 0:
                        aux_mm(m - 1)

                # top-8 logits + indices straight off PSUM, and the row
                # gathers they gate — before the softmax-sum chain, so
                # every round's gather DMA launches as early as possible
                t8 = work.tile([P, 8], f32, tag="t8")
                i8 = work.tile([P, 8], u32, tag="i8")
                nc.vector.max(out=t8[:], in_=lg[:])
                nc.vector.max_index(out=i8[:], in_max=t8[:], in_values=lg[:])
                nc.sync.dma_start(idx_d[m * P:(m + 1) * P, :], i8[:, 0:K])

                g0 = work.tile([P, D], f32, tag="g0")
                g1 = work.tile([P, D], f32, tag="g1")
                nc.gpsimd.indirect_dma_start(
                    out=g0[:], out_offset=None, in_=x_d[:],
                    in_offset=bass.IndirectOffsetOnAxis(ap=i8[:, 0:1], axis=0))
                nc.gpsimd.indirect_dma_start(
                    out=g1[:], out_offset=None, in_=x_d[:],
                    in_offset=bass.IndirectOffsetOnAxis(ap=i8[:, 1:2], axis=0))

                # u = exp(logits) (bf16) + fp32 row sums
                nc.scalar.activation(u_bf[:, m, :], lg[:],
                                     mybir.ActivationFunctionType.Exp,
                                     accum_out=acc[:, m:m + 1])
                nc.vector.reciprocal(rc[:, m:m + 1], acc[:, m:m + 1])
                # bf16 copy for the aux ones-matmul weights, produced on
                # DVE right behind the fp32 reciprocal so the interleaved
                # aux matmul never waits on a late-positioned ScalarE op
                nc.vector.tensor_copy(rc_bf[:, m:m + 1], rc[:, m:m + 1])

                # p_k = exp(top_k) / sum
                p01 = work.tile([P, K], f32, tag="p01")
                nc.scalar.activation(p01[:], t8[:, 0:K],
                                     mybir.ActivationFunctionType.Exp)
                nc.vector.tensor_scalar_mul(p01[:], p01[:], rc[:, m:m + 1])

                # three-tile-deep pipeline: gathers get ~3 matmul rounds of
                # slack before anything on a compute engine waits on them
                pending.append((g0, g1, p01, m))
                if len(pending) > 3:
                    combine(pending.pop(0))

            aux_mm(T - 1)
            for st in pending:
                combine(st)
            ps_sb = stats.tile([1, E], f32)
            nc.vector.tensor_copy(ps_sb[:], aux_ps[:])
            nc.sync.dma_start(ps_d[:], ps_sb[:])

    nc.finalize()
    return nc


_NC = None


def _split_T(a):
    """Return (hi, lo) bf16 split of a.T (fp32 [R, C] -> two [C, R])."""
    at = np.ascontiguousarray(a.T)
    hi = at.astype(ml_dtypes.bfloat16)
    lo = (at - hi.astype(np.float32)).astype(ml_dtypes.bfloat16)
    return hi, lo


def _run(x, W, **kw):
    global _NC
    if _NC is None:
        _NC = _build()
    x = np.ascontiguousarray(np.asarray(x, dtype=np.float32))
    W = np.ascontiguousarray(np.asarray(W, dtype=np.float32))
    wh, wl = _split_T(W)
    in_maps = []
    for c in range(NCORES):
        xh, xl = _split_T(x[c])
        in_maps.append({"x": x[c], "xh": xh, "xl": xl, "wh": wh, "wl": wl})
    return bass_utils.run_bass_kernel_spmd(
        _NC, in_maps, core_ids=list(range(NCORES)), **kw)


def kernel(x, W):
    res = _run(x, W)
    outs = res.results
    y = np.stack([outs[c]["y"] for c in range(NCORES)], axis=0)

    probs_sum = np.zeros(E, dtype=np.float64)
    idx_all = []
    for c in range(NCORES):
        probs_sum += outs[c]["ps"].reshape(E).astype(np.float64)
        idx_all.append(outs[c]["idx2"].reshape(-1))
    router_probs = (probs_sum / (B * S)).astype(np.float32)
    counts = np.bincount(np.concatenate(idx_all), minlength=E).astype(np.float32)
    router_fraction = counts / counts.sum()
    aux = np.float32(ALPHA * E * np.sum(router_probs * router_fraction,
                                        dtype=np.float64))
    return y, aux
